# revision 1
# baseline (speedup 1.0000x reference)
"""Trainium2 Bass kernel for nn_Char_30322469110372 (retrieval_knn).

Reference computation (per query b):
  ce   = row-normalized ce_raw (+ zero pad row for index -1)
  q    = ce[qidx[b]]
  for side in (l, r):
    u_side      = W_side @ q                     # [C]
    score[k]    = ce[ixs_c[b,k]] . u_side        # masked to -1e30 where ixs==-1
    attn        = softmax(score)
    emb_side    = sum_k attn[k] * wvec[ixs_w[b,k]]
  gate = softmax([attn_l, attn_r] @ gL_w.T + gL_b)
  out  = gate[0]*emb_l + gate[1]*emb_r

Sharding: data-parallel over B across 8 cores; ce/wvec tables replicated.

Device algorithm per core (B_core=2048 queries, 16 tiles of 128 queries,
processed in chunks of 4 tiles so gather DMA, DVE, ACT and Pool overlap):
  - normalization folded into scores: score = (ctx_raw.u_raw) * rctx * rq with
    rctx/rq = 1/max(||row||,1e-12) computed on gathered rows only.
  - scores are bounded (|score| <= ~1.2) so softmax needs no max-shift;
    exp(-1e30) underflows to exactly 0 for pad slots.
  - per tile: indirect-DMA gather of 18 ce rows/query; PE transposes q and
    computes u = [qT]^T @ [lW^T | rW^T]; DVE does the 17 dot products; ACT
    squares rows, DVE reduces to row sum-squares.
  - per chunk: softmax + gate pipeline on [128, 4*17] staging buffers.
  - per tile: gather 17 wvec rows/query (pad slots clamped to row 0; their
    softmax weight is exactly 0), then a 17-step scalar*tensor+tensor
    accumulation chain on DVE.

Note: indirect-DMA gathers use one [128,1]-index instruction per slot —
multi-index offset APs corrupt on HW through this runtime, and dma_gather
(int16 Q7 gather) hangs (its GPSIMD library never loads under axon/PJRT),
so ~2us/gather-instruction on the Pool engine is the binding constraint.
"""

from contextlib import ExitStack

import numpy as np

import concourse.bacc as bacc
import concourse.bass as bass
import concourse.mybir as mybir
import concourse.tile as tile
from concourse.bass_utils import run_bass_kernel_spmd
from concourse.masks import make_identity

# Problem shapes (hardcoded per contest contract).
P = 128
CD = 100          # char-embedding dim
L, R = 7, 10
K = L + R         # 17 context slots per query
KQ = K + 1        # + the query row itself
NCE = 200000      # ce table rows
V = 200000        # wvec table rows
WD = 300          # word-vector dim
B = 16384
N_CORES = 8
BC = B // N_CORES     # 2048 queries per core
NT = BC // P          # 16 tiles of 128 queries
CHUNK = 4             # tiles per phase chunk
OOB = 1 << 22         # stand-in index for -1; fails the DMA bounds check
N_POOL_K = 5          # wv accumulation steps offloaded to GPSIMD
WVP_BUFS = 3          # wv gather tile pool depth

F32 = mybir.dt.float32
I32 = mybir.dt.int32
Alu = mybir.AluOpType
Act = mybir.ActivationFunctionType
Ax = mybir.AxisListType


def _build_nc():
    nc = bacc.Bacc("TRN2", target_bir_lowering=False, debug=False,
                   num_devices=N_CORES)

    ce = nc.dram_tensor("ce_raw", [NCE, CD], F32, kind="ExternalInput")
    wv = nc.dram_tensor("wvec", [V, WD], F32, kind="ExternalInput")
    lW = nc.dram_tensor("lW", [CD, CD], F32, kind="ExternalInput")
    rW = nc.dram_tensor("rW", [CD, CD], F32, kind="ExternalInput")
    gw = nc.dram_tensor("gL_w", [2, K], F32, kind="ExternalInput")
    gb = nc.dram_tensor("gL_b", [2], F32, kind="ExternalInput")
    qidx = nc.dram_tensor("qidx", [BC], I32, kind="ExternalInput")
    lic = nc.dram_tensor("lixs_c", [BC, L], I32, kind="ExternalInput")
    ric = nc.dram_tensor("rixs_c", [BC, R], I32, kind="ExternalInput")
    liw = nc.dram_tensor("lixs_w", [BC, L], I32, kind="ExternalInput")
    riw = nc.dram_tensor("rixs_w", [BC, R], I32, kind="ExternalInput")
    out = nc.dram_tensor("out", [BC, WD], F32, kind="ExternalOutput")

    with tile.TileContext(nc) as tc, ExitStack() as ctx:
        consts = ctx.enter_context(tc.tile_pool(name="consts", bufs=1))
        stage = ctx.enter_context(tc.tile_pool(name="stage", bufs=1))
        cep = ctx.enter_context(tc.tile_pool(name="cep", bufs=4))
        wvp = ctx.enter_context(tc.tile_pool(name="wvp", bufs=WVP_BUFS))
        work = ctx.enter_context(tc.tile_pool(name="work", bufs=2))
        psum = ctx.enter_context(tc.tile_pool(name="psum", bufs=2, space="PSUM"))

        # ---------------- constants ----------------
        identity = consts.tile([P, P], F32)
        make_identity(nc, identity[:, :])

        # W^T for both sides packed as [100, 0:100]=lW^T, [100, 100:200]=rW^T
        wt_both = consts.tile([P, 2 * CD], F32)
        for side, wdram in enumerate((lW, rW)):
            wl = consts.tile([P, P], F32, name=f"wload{side}")
            nc.sync.dma_start(out=wl[0:CD, 0:CD], in_=wdram[:, :])
            wt_ps = psum.tile([P, P], F32, name=f"wt_ps{side}", tag="wt_ps")
            nc.tensor.transpose(
                out=wt_ps[0:CD, 0:CD], in_=wl[0:CD, 0:CD],
                identity=identity[0:CD, 0:CD])
            nc.vector.tensor_copy(
                out=wt_both[0:CD, side * CD:(side + 1) * CD],
                in_=wt_ps[0:CD, 0:CD])

        # gate weights replicated across partitions via PE outer product
        # (ones[128,1] @ row[1,36]); gwrep[:, j*K+k] = gL_w[j,k], cols 34:36=gL_b
        gwrow = consts.tile([1, 2 * K + 2], F32)
        nc.sync.dma_start(out=gwrow[0:1, 0:2 * K], in_=gw[:, :])
        nc.sync.dma_start(out=gwrow[0:1, 2 * K:2 * K + 2], in_=gb[:])
        ones1 = consts.tile([1, P], F32)
        nc.gpsimd.memset(ones1[:, :], 1.0)
        rep_ps = psum.tile([P, 2 * K + 2], F32, tag="rep_ps")
        nc.tensor.matmul(out=rep_ps[:, :], lhsT=ones1[0:1, :],
                         rhs=gwrow[0:1, :], start=True, stop=True)
        gwrep = consts.tile([P, 2 * K + 2], F32)
        nc.vector.tensor_copy(out=gwrep[:, :], in_=rep_ps[:, :])
        gbd = consts.tile([P, 1], F32)
        nc.vector.tensor_tensor(
            out=gbd[:, :], in0=gwrep[:, 2 * K + 1:2 * K + 2],
            in1=gwrep[:, 2 * K:2 * K + 1], op=Alu.subtract)

        # ---------------- index staging ----------------
        # cidx[p, t, 0] = qidx, [p, t, 1:8] = lixs_c, [p, t, 8:18] = rixs_c
        cidx = stage.tile([P, NT, KQ], I32)
        nc.sync.dma_start(out=cidx[:, :, 0],
                          in_=qidx[:].rearrange("(t p) -> p t", p=P))
        nc.sync.dma_start(out=cidx[:, :, 1:1 + L],
                          in_=lic[:, :].rearrange("(t p) k -> p t k", p=P))
        nc.sync.dma_start(out=cidx[:, :, 1 + L:KQ],
                          in_=ric[:, :].rearrange("(t p) k -> p t k", p=P))
        widx = stage.tile([P, NT, K], I32)
        nc.sync.dma_start(out=widx[:, :, 0:L],
                          in_=liw[:, :].rearrange("(t p) k -> p t k", p=P))
        nc.sync.dma_start(out=widx[:, :, L:K],
                          in_=riw[:, :].rearrange("(t p) k -> p t k", p=P))

        # clamped ce indices (pad -> row 0; masked later)
        ccl = stage.tile([P, NT, KQ], I32)
        nc.vector.tensor_scalar(out=ccl[:, :, :], in0=cidx[:, :, :],
                                scalar1=0, scalar2=None, op0=Alu.max)
        # wv indices: first WVP_BUFS tiles clamped (fully populate the pool
        # slots), later tiles map -1 -> OOB so the DMA bounds-check skips them
        wcl = stage.tile([P, NT, K], I32)
        nc.vector.tensor_scalar(out=wcl[:, :, :], in0=widx[:, :, :],
                                scalar1=0, scalar2=None, op0=Alu.max)
        wsk = stage.tile([P, NT, K], I32)
        nc.vector.tensor_scalar(out=wsk[:, :, :], in0=widx[:, :, :],
                                scalar1=0, scalar2=-OOB,
                                op0=Alu.min, op1=Alu.mult)
        nc.vector.tensor_tensor(out=wsk[:, :, :], in0=wsk[:, :, :],
                                in1=widx[:, :, :], op=Alu.add)

        # additive score mask: 0 for valid slots, -1e30 where index == -1
        maskf = stage.tile([P, NT, KQ], F32)
        nc.vector.tensor_copy(out=maskf[:, :, :], in_=cidx[:, :, :])
        maskt = stage.tile([P, NT, KQ], F32)
        nc.vector.tensor_scalar(out=maskt[:, :, :], in0=maskf[:, :, :],
                                scalar1=0.0, scalar2=1e30,
                                op0=Alu.min, op1=Alu.mult)

        # ---------------- staging buffers ----------------
        dotraw = stage.tile([P, NT, K], F32)
        cssq = stage.tile([P, NT, K], F32)
        qssq = stage.tile([P, NT], F32)
        expv = stage.tile([P, NT, K], F32)
        wall = stage.tile([P, NT, K], F32)

        def phase1_tile(t):
            ceg = cep.tile([P, KQ, CD], F32, name="ceg", tag="ceg")
            for s in range(KQ):
                nc.gpsimd.indirect_dma_start(
                    out=ceg[:, s, :], out_offset=None,
                    in_=ce[:, :],
                    in_offset=bass.IndirectOffsetOnAxis(
                        ap=ccl[:, t, s:s + 1], axis=0),
                )
            # transpose q rows: qt = ceg[:, 0, :]^T  -> [CD, P]
            qt_ps = psum.tile([P, P], F32, name="qt_ps", tag="qt_ps")
            nc.tensor.transpose(out=qt_ps[0:CD, :], in_=ceg[:, 0, :],
                                identity=identity[:, :])
            qt_sb = work.tile([P, P], F32, name="qt_sb", tag="qt_sb")
            nc.scalar.copy(out=qt_sb[0:CD, :], in_=qt_ps[0:CD, :])

            # u[q, 0:100] = lW @ q ; u[q, 100:200] = rW @ q
            u_ps = psum.tile([P, 2 * CD], F32, name="u_ps", tag="u_ps")
            nc.tensor.matmul(out=u_ps[:, :], lhsT=qt_sb[0:CD, :],
                             rhs=wt_both[0:CD, :], start=True, stop=True)

            # dot products: prod[p,k,c] = ctx[p,k,c] * u_side(k)[p,c]
            prod = work.tile([P, K, CD], F32, name="prod", tag="prod")
            nc.vector.tensor_tensor(
                out=prod[:, 0:L, :], in0=ceg[:, 1:1 + L, :],
                in1=u_ps[:, 0:CD].unsqueeze(1).broadcast_to((P, L, CD)),
                op=Alu.mult)
            nc.vector.tensor_tensor(
                out=prod[:, L:K, :], in0=ceg[:, 1 + L:KQ, :],
                in1=u_ps[:, CD:2 * CD].unsqueeze(1).broadcast_to((P, R, CD)),
                op=Alu.mult)
            nc.vector.tensor_reduce(out=dotraw[:, t, :], in_=prod[:, :, :],
                                    axis=Ax.X, op=Alu.add)

            # row sum-squares: ACT squares the whole gathered block (q+ctx),
            # DVE reduces each row
            sq = work.tile([P, KQ, CD], F32, name="sq", tag="sq")
            nc.scalar.activation(out=sq[:, :, :], in_=ceg[:, :, :],
                                 func=Act.Square)
            nc.vector.tensor_reduce(out=qssq[:, t:t + 1], in_=sq[:, 0, :],
                                    axis=Ax.X, op=Alu.add)
            nc.vector.tensor_reduce(out=cssq[:, t, :], in_=sq[:, 1:, :],
                                    axis=Ax.X, op=Alu.add)

        def phase2_chunk(t0, t1):
            n = t1 - t0
            ts = slice(t0, t1)
            # 1/max(||row||, 1e-12) for q and ctx rows
            sq_t = stage.tile([P, n], F32, name=f"sqt{t0}")
            nc.scalar.activation(out=sq_t[:, :], in_=qssq[:, ts],
                                 func=Act.Sqrt)
            nc.vector.tensor_scalar(out=sq_t[:, :], in0=sq_t[:, :],
                                    scalar1=1e-12, scalar2=None, op0=Alu.max)
            rq = stage.tile([P, n], F32, name=f"rq{t0}")
            nc.vector.reciprocal(out=rq[:, :], in_=sq_t[:, :])

            csq_t = stage.tile([P, n, K], F32, name=f"csqt{t0}")
            nc.scalar.activation(out=csq_t[:, :, :], in_=cssq[:, ts, :],
                                 func=Act.Sqrt)
            nc.vector.tensor_scalar(out=csq_t[:, :, :], in0=csq_t[:, :, :],
                                    scalar1=1e-12, scalar2=None, op0=Alu.max)
            rctx = stage.tile([P, n, K], F32, name=f"rctx{t0}")
            nc.vector.reciprocal(out=rctx[:, :, :], in_=csq_t[:, :, :])

            scr = stage.tile([P, n, K], F32, name=f"scr{t0}")
            nc.vector.tensor_tensor(out=scr[:, :, :], in0=dotraw[:, ts, :],
                                    in1=rctx[:, :, :], op=Alu.mult)
            nc.vector.tensor_tensor(
                out=scr[:, :, :], in0=scr[:, :, :],
                in1=rq[:, :].unsqueeze(2).broadcast_to((P, n, K)),
                op=Alu.mult)
            nc.vector.tensor_tensor(out=scr[:, :, :], in0=scr[:, :, :],
                                    in1=maskt[:, ts, 1:KQ], op=Alu.add)

            nc.scalar.activation(out=expv[:, ts, :], in_=scr[:, :, :],
                                 func=Act.Exp)

            sum_l = stage.tile([P, n], F32, name=f"suml{t0}")
            nc.vector.tensor_reduce(out=sum_l[:, :], in_=expv[:, ts, 0:L],
                                    axis=Ax.X, op=Alu.add)
            sum_r = stage.tile([P, n], F32, name=f"sumr{t0}")
            nc.vector.tensor_reduce(out=sum_r[:, :], in_=expv[:, ts, L:K],
                                    axis=Ax.X, op=Alu.add)
            rs_l = stage.tile([P, n], F32, name=f"rsl{t0}")
            nc.vector.reciprocal(out=rs_l[:, :], in_=sum_l[:, :])
            rs_r = stage.tile([P, n], F32, name=f"rsr{t0}")
            nc.vector.reciprocal(out=rs_r[:, :], in_=sum_r[:, :])

            # gate logit difference dz = (z1-z0) + (gb1-gb0), where
            # z_j = rs_l * sum_k exp_l[k] gw[j,k] + rs_r * sum_k exp_r[k] gw[j,..]
            d = {}
            gtmp_l = stage.tile([P, n, L], F32, name=f"gtl{t0}")
            gtmp_r = stage.tile([P, n, R], F32, name=f"gtr{t0}")
            for j in (0, 1):
                nc.vector.tensor_tensor(
                    out=gtmp_l[:, :, :], in0=expv[:, ts, 0:L],
                    in1=gwrep[:, j * K:j * K + L].unsqueeze(1)
                        .broadcast_to((P, n, L)),
                    op=Alu.mult)
                d[j, 'l'] = stage.tile([P, n], F32, name=f"d{j}l{t0}")
                nc.vector.tensor_reduce(out=d[j, 'l'][:, :],
                                        in_=gtmp_l[:, :, :],
                                        axis=Ax.X, op=Alu.add)
                nc.vector.tensor_tensor(
                    out=gtmp_r[:, :, :], in0=expv[:, ts, L:K],
                    in1=gwrep[:, j * K + L:(j + 1) * K].unsqueeze(1)
                        .broadcast_to((P, n, R)),
                    op=Alu.mult)
                d[j, 'r'] = stage.tile([P, n], F32, name=f"d{j}r{t0}")
                nc.vector.tensor_reduce(out=d[j, 'r'][:, :],
                                        in_=gtmp_r[:, :, :],
                                        axis=Ax.X, op=Alu.add)

            ddl = stage.tile([P, n], F32, name=f"ddl{t0}")
            nc.vector.tensor_tensor(out=ddl[:, :], in0=d[1, 'l'][:, :],
                                    in1=d[0, 'l'][:, :], op=Alu.subtract)
            ddr = stage.tile([P, n], F32, name=f"ddr{t0}")
            nc.vector.tensor_tensor(out=ddr[:, :], in0=d[1, 'r'][:, :],
                                    in1=d[0, 'r'][:, :], op=Alu.subtract)
            m1 = stage.tile([P, n], F32, name=f"m1{t0}")
            nc.vector.tensor_tensor(out=m1[:, :], in0=ddl[:, :],
                                    in1=rs_l[:, :], op=Alu.mult)
            m2 = stage.tile([P, n], F32, name=f"m2{t0}")
            nc.vector.tensor_tensor(out=m2[:, :], in0=ddr[:, :],
                                    in1=rs_r[:, :], op=Alu.mult)
            dz = stage.tile([P, n], F32, name=f"dz{t0}")
            nc.vector.tensor_tensor(out=dz[:, :], in0=m1[:, :], in1=m2[:, :],
                                    op=Alu.add)
            nc.vector.tensor_scalar(out=dz[:, :], in0=dz[:, :],
                                    scalar1=gbd[:, 0:1], scalar2=None,
                                    op0=Alu.add)

            e1 = stage.tile([P, n], F32, name=f"e1{t0}")
            nc.scalar.activation(out=e1[:, :], in_=dz[:, :], func=Act.Exp)
            den = stage.tile([P, n], F32, name=f"den{t0}")
            nc.vector.tensor_scalar(out=den[:, :], in0=e1[:, :], scalar1=1.0,
                                    scalar2=None, op0=Alu.add)
            rden = stage.tile([P, n], F32, name=f"rden{t0}")
            nc.vector.reciprocal(out=rden[:, :], in_=den[:, :])

            # c_l = g0*rs_l = rs_l/(1+e1); c_r = g1*rs_r = rs_r*e1/(1+e1)
            c_l = stage.tile([P, n], F32, name=f"cl{t0}")
            nc.vector.tensor_tensor(out=c_l[:, :], in0=rs_l[:, :],
                                    in1=rden[:, :], op=Alu.mult)
            c_r = stage.tile([P, n], F32, name=f"cr{t0}")
            nc.vector.tensor_tensor(out=c_r[:, :], in0=rs_r[:, :],
                                    in1=rden[:, :], op=Alu.mult)
            nc.vector.tensor_tensor(out=c_r[:, :], in0=c_r[:, :],
                                    in1=e1[:, :], op=Alu.mult)

            # final per-slot weights
            nc.vector.tensor_tensor(
                out=wall[:, ts, 0:L], in0=expv[:, ts, 0:L],
                in1=c_l[:, :].unsqueeze(2).broadcast_to((P, n, L)),
                op=Alu.mult)
            nc.vector.tensor_tensor(
                out=wall[:, ts, L:K], in0=expv[:, ts, L:K],
                in1=c_r[:, :].unsqueeze(2).broadcast_to((P, n, R)),
                op=Alu.mult)

        def phase3_tile(t):
            wvg = wvp.tile([P, K, WD], F32, name="wvg", tag="wvg")
            for s in range(K):
                if t < WVP_BUFS:
                    # clamped indices fully populate the pool slot
                    nc.gpsimd.indirect_dma_start(
                        out=wvg[:, s, :], out_offset=None,
                        in_=wv[:, :],
                        in_offset=bass.IndirectOffsetOnAxis(
                            ap=wcl[:, t, s:s + 1], axis=0),
                    )
                else:
                    # pad slots carry OOB indices -> descriptor skipped; the
                    # stale slot data is finite and weighted by exactly 0
                    nc.gpsimd.indirect_dma_start(
                        out=wvg[:, s, :], out_offset=None,
                        in_=wv[:, :],
                        in_offset=bass.IndirectOffsetOnAxis(
                            ap=wsk[:, t, s:s + 1], axis=0),
                        bounds_check=V - 1, oob_is_err=False,
                    )
            acc_a = work.tile([P, WD], F32, name="acc_a", tag="acc_a")
            acc_b = work.tile([P, WD], F32, name="acc_b", tag="acc_b")
            nc.vector.tensor_scalar(out=acc_a[:, :], in0=wvg[:, 0, :],
                                    scalar1=wall[:, t, 0:1], scalar2=None,
                                    op0=Alu.mult)
            for k in range(1, K):
                src, dst = (acc_a, acc_b) if k % 2 == 1 else (acc_b, acc_a)
                nc.vector.scalar_tensor_tensor(
                    out=dst[:, :], in0=wvg[:, k, :],
                    scalar=wall[:, t, k:k + 1], in1=src[:, :],
                    op0=Alu.mult, op1=Alu.add)
            res = acc_a if (K - 1) % 2 == 0 else acc_b
            nc.sync.dma_start(out=out[t * P:(t + 1) * P, :], in_=res[:, :])

        for c0 in range(0, NT, CHUNK):
            for t in range(c0, c0 + CHUNK):
                phase1_tile(t)
            phase2_chunk(c0, c0 + CHUNK)
            for t in range(c0, c0 + CHUNK):
                phase3_tile(t)

    nc.compile()
    return nc


_NC_CACHE = None


def _get_nc():
    global _NC_CACHE
    if _NC_CACHE is None:
        _NC_CACHE = _build_nc()
    return _NC_CACHE


def kernel(**inputs):
    inp = {k: np.asarray(v) for k, v in inputs.items()}
    nc = _get_nc()
    shared = {k: inp[k] for k in ("ce_raw", "wvec", "lW", "rW", "gL_w", "gL_b")}
    in_maps = []
    for c in range(N_CORES):
        sl = slice(c * BC, (c + 1) * BC)
        m = dict(shared)
        for name in ("qidx", "lixs_c", "rixs_c", "lixs_w", "rixs_w"):
            m[name] = np.ascontiguousarray(inp[name][sl])
        in_maps.append(m)
    res = run_bass_kernel_spmd(nc, in_maps, list(range(N_CORES)))
    return np.concatenate([res.results[c]["out"] for c in range(N_CORES)],
                          axis=0)



# revision 5
# speedup vs baseline: 1.4513x; 1.4513x over previous
"""Trainium2 Bass kernel for nn_Char_30322469110372 (retrieval_knn).

Reference computation (per query b):
  ce   = row-normalized ce_raw (+ zero pad row for index -1)
  q    = ce[qidx[b]]
  for side in (l, r):
    u_side      = W_side @ q                     # [C]
    score[k]    = ce[ixs_c[b,k]] . u_side        # masked to -1e30 where ixs==-1
    attn        = softmax(score)
    emb_side    = sum_k attn[k] * wvec[ixs_w[b,k]]
  gate = softmax([attn_l, attn_r] @ gL_w.T + gL_b)
  out  = gate[0]*gate_l + gate[1]*emb_r

Sharding: data-parallel over B across 8 cores; ce/wvec tables replicated.

The binding constraint is the Pool engine: every indirect-DMA gather costs
~1us of SWDGE descriptor-generation time regardless of payload, and
multi-index offset APs corrupt/crash on HW through this runtime, so one
gather instruction moves at most 128 rows (one per partition).  The win
here over the naive per-slot loop: queries are SORTED by their valid
context counts (llen, rlen) on the host and dealt to (core, tile) slots so
that each tile only issues gather instructions for slots that are valid
for at least one of its 128 queries.  Sorted-contiguous dealing gives each
program tile j a shared slot budget (bl[j], br[j]) across all 8 cores;
slots beyond the budget are pad (-1) for every query in the tile, carry an
additive -1e30 mask, and are never gathered (instruction skipped).  This
cuts gather instructions from 35/tile to 1+2*(bl+br) with bl+br ~ 11.5.

Device algorithm per core (2048 queries, 16 tiles of 128, chunks of 4):
  - normalization folded into scores: score = (ctx_raw.u_raw) * rctx * rq.
  - scores bounded so softmax needs no max-shift; exp(-1e30) == 0 exactly.
  - phase1/tile: gather 1+bl+br ce rows/query; PE transposes q, computes
    u = [qT]^T @ [lW^T | rW^T]; DVE dot products; ACT squares; DVE reduces
    row sum-squares.
  - phase2/chunk: softmax + gate on [128, 4*17] staging; l/r slot ranges
    vary per tile (l at 0..bl, r at bl..bl+br) so the per-side reductions
    are per-tile; the gate logit difference uses a host-packed per-tile
    selection of gL_w columns (device computes row1-row0 and replicates).
  - phase3/tile: gather bl+br wvec rows/query (pad slots clamped to row 0,
    weight exactly 0), then scalar*tensor+tensor accumulation on DVE.
"""

from contextlib import ExitStack

import numpy as np

import concourse.bacc as bacc
import concourse.bass as bass
import concourse.mybir as mybir
import concourse.tile as tile
from concourse.bass_utils import run_bass_kernel_spmd
from concourse.masks import make_identity

# Problem shapes (hardcoded per contest contract).
P = 128
CD = 100          # char-embedding dim
L, R = 7, 10
K = L + R         # 17 context slots per query
KQ = K + 1        # + the query row itself
NCE = 200000      # ce table rows
V = 200000        # wvec table rows
WD = 300          # word-vector dim
B = 16384
N_CORES = 8
BC = B // N_CORES     # 2048 queries per core
NT = BC // P          # 16 tiles of 128 queries
CHUNK = 4             # tiles per phase chunk

F32 = mybir.dt.float32
I32 = mybir.dt.int32
Alu = mybir.AluOpType
Act = mybir.ActivationFunctionType
Ax = mybir.AxisListType


def _build_nc(bl, br):
    """bl/br: per-tile l/r slot budgets (length NT tuples)."""
    nc = bacc.Bacc("TRN2", target_bir_lowering=False, debug=False,
                   num_devices=N_CORES)

    ce = nc.dram_tensor("ce_raw", [NCE, CD], F32, kind="ExternalInput")
    wv = nc.dram_tensor("wvec", [V, WD], F32, kind="ExternalInput")
    lW = nc.dram_tensor("lW", [CD, CD], F32, kind="ExternalInput")
    rW = nc.dram_tensor("rW", [CD, CD], F32, kind="ExternalInput")
    gsel = nc.dram_tensor("gsel", [2, NT * K], F32, kind="ExternalInput")
    gb = nc.dram_tensor("gL_b", [2], F32, kind="ExternalInput")
    # host-packed per-(partition, tile, slot) index tensors
    cidx_d = nc.dram_tensor("cidx", [P, NT, KQ], I32, kind="ExternalInput")
    widx_d = nc.dram_tensor("widx", [P, NT, K], I32, kind="ExternalInput")
    out = nc.dram_tensor("out", [BC, WD], F32, kind="ExternalOutput")

    with tile.TileContext(nc) as tc, ExitStack() as ctx:
        consts = ctx.enter_context(tc.tile_pool(name="consts", bufs=1))
        stage = ctx.enter_context(tc.tile_pool(name="stage", bufs=1))
        cep = ctx.enter_context(tc.tile_pool(name="cep", bufs=4))
        wvp = ctx.enter_context(tc.tile_pool(name="wvp", bufs=3))
        work = ctx.enter_context(tc.tile_pool(name="work", bufs=2))
        psum = ctx.enter_context(tc.tile_pool(name="psum", bufs=2, space="PSUM"))

        # ---------------- constants ----------------
        identity = consts.tile([P, P], F32)
        make_identity(nc, identity[:, :])

        # W^T for both sides packed as [100, 0:100]=lW^T, [100, 100:200]=rW^T
        wt_both = consts.tile([P, 2 * CD], F32)
        for side, wdram in enumerate((lW, rW)):
            wl = consts.tile([P, P], F32, name=f"wload{side}")
            nc.sync.dma_start(out=wl[0:CD, 0:CD], in_=wdram[:, :])
            wt_ps = psum.tile([P, P], F32, name=f"wt_ps{side}", tag="wt_ps")
            nc.tensor.transpose(
                out=wt_ps[0:CD, 0:CD], in_=wl[0:CD, 0:CD],
                identity=identity[0:CD, 0:CD])
            nc.vector.tensor_copy(
                out=wt_both[0:CD, side * CD:(side + 1) * CD],
                in_=wt_ps[0:CD, 0:CD])

        # per-tile gate-weight difference, replicated across partitions:
        # gwd[p, t*K+s] = gsel[1, t*K+s] - gsel[0, t*K+s]
        gselrow = consts.tile([1, 2 * NT * K + 2], F32)
        nc.sync.dma_start(out=gselrow[0:1, 0:2 * NT * K], in_=gsel[:, :])
        nc.sync.dma_start(
            out=gselrow[0:1, 2 * NT * K:2 * NT * K + 2], in_=gb[:])
        gdrow = consts.tile([1, NT * K + 1], F32)
        nc.vector.tensor_tensor(
            out=gdrow[0:1, 0:NT * K], in0=gselrow[0:1, NT * K:2 * NT * K],
            in1=gselrow[0:1, 0:NT * K], op=Alu.subtract)
        nc.vector.tensor_tensor(
            out=gdrow[0:1, NT * K:NT * K + 1],
            in0=gselrow[0:1, 2 * NT * K + 1:2 * NT * K + 2],
            in1=gselrow[0:1, 2 * NT * K:2 * NT * K + 1], op=Alu.subtract)
        ones1 = consts.tile([1, P], F32)
        nc.vector.memset(ones1[:, :], 1.0)
        gwdrep = consts.tile([P, NT, K], F32)
        gbd = consts.tile([P, 1], F32)
        # replicate in two matmuls (PSUM bank is 512 f32 wide)
        half = (NT // 2) * K
        for h in range(2):
            cols = slice(h * half, (h + 1) * half)
            rep_ps = psum.tile([P, half], F32, name=f"rep{h}", tag="rep_ps")
            nc.tensor.matmul(out=rep_ps[:, :], lhsT=ones1[0:1, :],
                             rhs=gdrow[0:1, cols], start=True, stop=True)
            nc.vector.tensor_copy(
                out=gwdrep[:, h * (NT // 2):(h + 1) * (NT // 2), :]
                    .rearrange("p a b -> p (a b)"),
                in_=rep_ps[:, :])
        rep_ps = psum.tile([P, 1], F32, name="repb", tag="rep_ps")
        nc.tensor.matmul(out=rep_ps[:, :], lhsT=ones1[0:1, :],
                         rhs=gdrow[0:1, NT * K:NT * K + 1],
                         start=True, stop=True)
        nc.vector.tensor_copy(out=gbd[:, :], in_=rep_ps[:, :])

        # ---------------- index staging ----------------
        cidx = stage.tile([P, NT, KQ], I32)
        nc.sync.dma_start(out=cidx[:, :, :], in_=cidx_d[:, :, :])
        widx = stage.tile([P, NT, K], I32)
        nc.sync.dma_start(out=widx[:, :, :], in_=widx_d[:, :, :])

        # clamped indices (pad -> row 0; weight is exactly 0 downstream)
        ccl = stage.tile([P, NT, KQ], I32)
        nc.vector.tensor_scalar(out=ccl[:, :, :], in0=cidx[:, :, :],
                                scalar1=0, scalar2=None, op0=Alu.max)
        wcl = stage.tile([P, NT, K], I32)
        nc.vector.tensor_scalar(out=wcl[:, :, :], in0=widx[:, :, :],
                                scalar1=0, scalar2=None, op0=Alu.max)

        # additive score mask: 0 for valid slots, -1e30 where index == -1
        maskf = stage.tile([P, NT, KQ], F32)
        nc.vector.tensor_copy(out=maskf[:, :, :], in_=cidx[:, :, :])
        maskt = stage.tile([P, NT, KQ], F32)
        nc.vector.tensor_scalar(out=maskt[:, :, :], in0=maskf[:, :, :],
                                scalar1=0.0, scalar2=1e30,
                                op0=Alu.min, op1=Alu.mult)

        # ---------------- staging buffers ----------------
        # dotraw zero / cssq one so never-gathered slots stay finite
        dotraw = stage.tile([P, NT, K], F32)
        nc.vector.memset(dotraw[:, :, :], 0.0)
        cssq = stage.tile([P, NT, K], F32)
        nc.vector.memset(cssq[:, :, :], 1.0)
        qssq = stage.tile([P, NT], F32)
        expv = stage.tile([P, NT, K], F32)
        wall = stage.tile([P, NT, K], F32)

        def phase1_tile(t):
            nk = 1 + bl[t] + br[t]     # gathered ce rows per query
            ceg = cep.tile([P, KQ, CD], F32, name="ceg", tag="ceg")
            for s in range(nk):
                nc.gpsimd.indirect_dma_start(
                    out=ceg[:, s, :], out_offset=None,
                    in_=ce[:, :],
                    in_offset=bass.IndirectOffsetOnAxis(
                        ap=ccl[:, t, s:s + 1], axis=0),
                )
            # transpose q rows: qt = ceg[:, 0, :]^T  -> [CD, P]
            qt_ps = psum.tile([P, P], F32, name="qt_ps", tag="qt_ps")
            nc.tensor.transpose(out=qt_ps[0:CD, :], in_=ceg[:, 0, :],
                                identity=identity[:, :])
            qt_sb = work.tile([P, P], F32, name="qt_sb", tag="qt_sb")
            nc.scalar.copy(out=qt_sb[0:CD, :], in_=qt_ps[0:CD, :])

            # u[q, 0:100] = lW @ q ; u[q, 100:200] = rW @ q
            u_ps = psum.tile([P, 2 * CD], F32, name="u_ps", tag="u_ps")
            nc.tensor.matmul(out=u_ps[:, :], lhsT=qt_sb[0:CD, :],
                             rhs=wt_both[0:CD, :], start=True, stop=True)

            # dot products: prod[p,k,c] = ctx[p,k,c] * u_side(k)[p,c]
            # l slots at 1..1+bl, r slots at 1+bl..nk
            prod = work.tile([P, K, CD], F32, name="prod", tag="prod")
            nc.vector.tensor_tensor(
                out=prod[:, 0:bl[t], :], in0=ceg[:, 1:1 + bl[t], :],
                in1=u_ps[:, 0:CD].unsqueeze(1).broadcast_to((P, bl[t], CD)),
                op=Alu.mult)
            nc.vector.tensor_tensor(
                out=prod[:, bl[t]:nk - 1, :], in0=ceg[:, 1 + bl[t]:nk, :],
                in1=u_ps[:, CD:2 * CD].unsqueeze(1)
                    .broadcast_to((P, br[t], CD)),
                op=Alu.mult)
            nc.vector.tensor_reduce(out=dotraw[:, t, 0:nk - 1],
                                    in_=prod[:, 0:nk - 1, :],
                                    axis=Ax.X, op=Alu.add)

            # row sum-squares: ACT squares the gathered block, DVE reduces
            sq = work.tile([P, KQ, CD], F32, name="sq", tag="sq")
            nc.scalar.activation(out=sq[:, 0:nk, :], in_=ceg[:, 0:nk, :],
                                 func=Act.Square)
            nc.vector.tensor_reduce(out=qssq[:, t:t + 1], in_=sq[:, 0, :],
                                    axis=Ax.X, op=Alu.add)
            nc.vector.tensor_reduce(out=cssq[:, t, 0:nk - 1],
                                    in_=sq[:, 1:nk, :],
                                    axis=Ax.X, op=Alu.add)

        def phase2_chunk(t0, t1):
            n = t1 - t0
            ts = slice(t0, t1)
            # 1/max(||row||, 1e-12) for q and ctx rows
            sq_t = stage.tile([P, n], F32, name=f"sqt{t0}")
            nc.scalar.activation(out=sq_t[:, :], in_=qssq[:, ts],
                                 func=Act.Sqrt)
            nc.vector.tensor_scalar(out=sq_t[:, :], in0=sq_t[:, :],
                                    scalar1=1e-12, scalar2=None, op0=Alu.max)
            rq = stage.tile([P, n], F32, name=f"rq{t0}")
            nc.vector.reciprocal(out=rq[:, :], in_=sq_t[:, :])

            csq_t = stage.tile([P, n, K], F32, name=f"csqt{t0}")
            nc.scalar.activation(out=csq_t[:, :, :], in_=cssq[:, ts, :],
                                 func=Act.Sqrt)
            nc.vector.tensor_scalar(out=csq_t[:, :, :], in0=csq_t[:, :, :],
                                    scalar1=1e-12, scalar2=None, op0=Alu.max)
            rctx = stage.tile([P, n, K], F32, name=f"rctx{t0}")
            nc.vector.reciprocal(out=rctx[:, :, :], in_=csq_t[:, :, :])

            scr = stage.tile([P, n, K], F32, name=f"scr{t0}")
            nc.vector.tensor_tensor(out=scr[:, :, :], in0=dotraw[:, ts, :],
                                    in1=rctx[:, :, :], op=Alu.mult)
            nc.vector.tensor_tensor(
                out=scr[:, :, :], in0=scr[:, :, :],
                in1=rq[:, :].unsqueeze(2).broadcast_to((P, n, K)),
                op=Alu.mult)
            nc.vector.tensor_tensor(out=scr[:, :, :], in0=scr[:, :, :],
                                    in1=maskt[:, ts, 1:KQ], op=Alu.add)

            nc.scalar.activation(out=expv[:, ts, :], in_=scr[:, :, :],
                                 func=Act.Exp)

            # per-tile l/r reductions (slot ranges vary with budgets)
            sum_l = stage.tile([P, n], F32, name=f"suml{t0}")
            sum_r = stage.tile([P, n], F32, name=f"sumr{t0}")
            gmul = stage.tile([P, n, K], F32, name=f"gmul{t0}")
            nc.vector.tensor_tensor(out=gmul[:, :, :], in0=expv[:, ts, :],
                                    in1=gwdrep[:, ts, :], op=Alu.mult)
            ddl = stage.tile([P, n], F32, name=f"ddl{t0}")
            ddr = stage.tile([P, n], F32, name=f"ddr{t0}")
            for t in range(t0, t1):
                i = t - t0
                nc.vector.tensor_reduce(
                    out=sum_l[:, i:i + 1], in_=expv[:, t, 0:bl[t]],
                    axis=Ax.X, op=Alu.add)
                nc.vector.tensor_reduce(
                    out=sum_r[:, i:i + 1],
                    in_=expv[:, t, bl[t]:bl[t] + br[t]],
                    axis=Ax.X, op=Alu.add)
                nc.vector.tensor_reduce(
                    out=ddl[:, i:i + 1], in_=gmul[:, i, 0:bl[t]],
                    axis=Ax.X, op=Alu.add)
                nc.vector.tensor_reduce(
                    out=ddr[:, i:i + 1], in_=gmul[:, i, bl[t]:bl[t] + br[t]],
                    axis=Ax.X, op=Alu.add)
            rs_l = stage.tile([P, n], F32, name=f"rsl{t0}")
            nc.vector.reciprocal(out=rs_l[:, :], in_=sum_l[:, :])
            rs_r = stage.tile([P, n], F32, name=f"rsr{t0}")
            nc.vector.reciprocal(out=rs_r[:, :], in_=sum_r[:, :])

            # gate logit difference dz = ddl*rs_l + ddr*rs_r + (gb1-gb0)
            m1 = stage.tile([P, n], F32, name=f"m1{t0}")
            nc.vector.tensor_tensor(out=m1[:, :], in0=ddl[:, :],
                                    in1=rs_l[:, :], op=Alu.mult)
            m2 = stage.tile([P, n], F32, name=f"m2{t0}")
            nc.vector.tensor_tensor(out=m2[:, :], in0=ddr[:, :],
                                    in1=rs_r[:, :], op=Alu.mult)
            dz = stage.tile([P, n], F32, name=f"dz{t0}")
            nc.vector.tensor_tensor(out=dz[:, :], in0=m1[:, :], in1=m2[:, :],
                                    op=Alu.add)
            nc.vector.tensor_scalar(out=dz[:, :], in0=dz[:, :],
                                    scalar1=gbd[:, 0:1], scalar2=None,
                                    op0=Alu.add)

            e1 = stage.tile([P, n], F32, name=f"e1{t0}")
            nc.scalar.activation(out=e1[:, :], in_=dz[:, :], func=Act.Exp)
            den = stage.tile([P, n], F32, name=f"den{t0}")
            nc.vector.tensor_scalar(out=den[:, :], in0=e1[:, :], scalar1=1.0,
                                    scalar2=None, op0=Alu.add)
            rden = stage.tile([P, n], F32, name=f"rden{t0}")
            nc.vector.reciprocal(out=rden[:, :], in_=den[:, :])

            # c_l = g0*rs_l = rs_l/(1+e1); c_r = g1*rs_r = rs_r*e1/(1+e1)
            c_l = stage.tile([P, n], F32, name=f"cl{t0}")
            nc.vector.tensor_tensor(out=c_l[:, :], in0=rs_l[:, :],
                                    in1=rden[:, :], op=Alu.mult)
            c_r = stage.tile([P, n], F32, name=f"cr{t0}")
            nc.vector.tensor_tensor(out=c_r[:, :], in0=rs_r[:, :],
                                    in1=rden[:, :], op=Alu.mult)
            nc.vector.tensor_tensor(out=c_r[:, :], in0=c_r[:, :],
                                    in1=e1[:, :], op=Alu.mult)

            # final per-slot weights (per-tile ranges)
            for t in range(t0, t1):
                i = t - t0
                nc.vector.tensor_tensor(
                    out=wall[:, t, 0:bl[t]], in0=expv[:, t, 0:bl[t]],
                    in1=c_l[:, i:i + 1].broadcast_to((P, bl[t])),
                    op=Alu.mult)
                nc.vector.tensor_tensor(
                    out=wall[:, t, bl[t]:bl[t] + br[t]],
                    in0=expv[:, t, bl[t]:bl[t] + br[t]],
                    in1=c_r[:, i:i + 1].broadcast_to((P, br[t])),
                    op=Alu.mult)

        def phase3_tile(t):
            nw = bl[t] + br[t]
            wvg = wvp.tile([P, K, WD], F32, name="wvg", tag="wvg")
            for s in range(nw):
                nc.gpsimd.indirect_dma_start(
                    out=wvg[:, s, :], out_offset=None,
                    in_=wv[:, :],
                    in_offset=bass.IndirectOffsetOnAxis(
                        ap=wcl[:, t, s:s + 1], axis=0),
                )
            acc_a = work.tile([P, WD], F32, name="acc_a", tag="acc_a")
            acc_b = work.tile([P, WD], F32, name="acc_b", tag="acc_b")
            nc.vector.tensor_scalar(out=acc_a[:, :], in0=wvg[:, 0, :],
                                    scalar1=wall[:, t, 0:1], scalar2=None,
                                    op0=Alu.mult)
            for s in range(1, nw):
                src, dst = (acc_a, acc_b) if s % 2 == 1 else (acc_b, acc_a)
                nc.vector.scalar_tensor_tensor(
                    out=dst[:, :], in0=wvg[:, s, :],
                    scalar=wall[:, t, s:s + 1], in1=src[:, :],
                    op0=Alu.mult, op1=Alu.add)
            res = acc_a if (nw - 1) % 2 == 0 else acc_b
            nc.sync.dma_start(out=out[t * P:(t + 1) * P, :], in_=res[:, :])

        for c0 in range(0, NT, CHUNK):
            for t in range(c0, c0 + CHUNK):
                phase1_tile(t)
            phase2_chunk(c0, c0 + CHUNK)
            for t in range(c0, c0 + CHUNK):
                phase3_tile(t)

    nc.compile()
    return nc


_NC_CACHE = {}


def _get_nc(bl, br):
    key = (tuple(bl), tuple(br))
    if key not in _NC_CACHE:
        _NC_CACHE[key] = _build_nc(tuple(bl), tuple(br))
    return _NC_CACHE[key]


def _plan(lixs_c, rixs_c):
    """Sort queries by (llen, rlen) snake order; deal sorted 128-query
    tiles round-robin to cores; budgets are per-program-tile maxima."""
    llen = (lixs_c != -1).sum(1)                      # [B]
    rlen = (rixs_c != -1).sum(1)
    key = (llen - 1) * 100 + np.where(llen % 2 == 1, rlen, R + 1 - rlen)
    order = np.argsort(key, kind="stable")            # [B] global query ids
    grp = order.reshape(NT, N_CORES * P)              # budget groups
    bl = llen[grp].max(1)                             # [NT]
    br = rlen[grp].max(1)
    # bsel[c, j, p] = global query id at (core c, tile j, partition p)
    bsel = order.reshape(NT, N_CORES, P).transpose(1, 0, 2)
    return bsel, bl.tolist(), br.tolist()


def prepare(inputs):
    """Plan, build the module and the per-core input maps (no run)."""
    inp = {k: np.asarray(v) for k, v in inputs.items()}
    bsel, bl, br = _plan(inp["lixs_c"], inp["rixs_c"])
    nc = _get_nc(bl, br)

    # per-tile selection of gL_w columns: l slots 0..bl-1 -> cols 0..bl-1,
    # r slots bl..bl+br-1 -> cols L..L+br-1, rest zero.
    gw = inp["gL_w"]
    gsel = np.zeros((2, NT, K), dtype=np.float32)
    for j in range(NT):
        gsel[:, j, 0:bl[j]] = gw[:, 0:bl[j]]
        gsel[:, j, bl[j]:bl[j] + br[j]] = gw[:, L:L + br[j]]
    gsel = np.ascontiguousarray(gsel.reshape(2, NT * K))

    shared = {k: inp[k] for k in ("ce_raw", "wvec", "lW", "rW", "gL_b")}
    shared["gsel"] = gsel
    in_maps = []
    for c in range(N_CORES):
        sel = bsel[c]                                  # [NT, P] query ids
        cidx = np.full((NT, P, KQ), -1, dtype=np.int32)
        cidx[:, :, 0] = inp["qidx"][sel]
        widx = np.full((NT, P, K), -1, dtype=np.int32)
        lc, rc = inp["lixs_c"][sel], inp["rixs_c"][sel]   # [NT, P, L/R]
        lw, rw = inp["lixs_w"][sel], inp["rixs_w"][sel]
        for j in range(NT):
            cidx[j, :, 1:1 + bl[j]] = lc[j, :, 0:bl[j]]
            cidx[j, :, 1 + bl[j]:1 + bl[j] + br[j]] = rc[j, :, 0:br[j]]
            widx[j, :, 0:bl[j]] = lw[j, :, 0:bl[j]]
            widx[j, :, bl[j]:bl[j] + br[j]] = rw[j, :, 0:br[j]]
        m = dict(shared)
        m["cidx"] = np.ascontiguousarray(cidx.transpose(1, 0, 2))
        m["widx"] = np.ascontiguousarray(widx.transpose(1, 0, 2))
        in_maps.append(m)
    return in_maps, nc, bsel


def kernel(**inputs):
    in_maps, nc, bsel = prepare(inputs)
    res = run_bass_kernel_spmd(nc, in_maps, list(range(N_CORES)))
    out = np.empty((B, WD), dtype=np.float32)
    for c in range(N_CORES):
        # device row j*P+p holds query bsel[c, j, p]
        out[bsel[c].reshape(-1)] = res.results[c]["out"]
    return out


# for test.py's cost-model timing fallback
def _last_nc():
    return next(iter(_NC_CACHE.values())) if _NC_CACHE else None


# revision 8
# speedup vs baseline: 1.5368x; 1.0589x over previous
"""Trainium2 Bass kernel for nn_Char_30322469110372 (retrieval_knn).

Reference computation (per query b):
  ce   = row-normalized ce_raw (+ zero pad row for index -1)
  q    = ce[qidx[b]]
  for side in (l, r):
    u_side      = W_side @ q                     # [C]
    score[k]    = ce[ixs_c[b,k]] . u_side        # masked to -1e30 where ixs==-1
    attn        = softmax(score)
    emb_side    = sum_k attn[k] * wvec[ixs_w[b,k]]
  gate = softmax([attn_l, attn_r] @ gL_w.T + gL_b)
  out  = gate[0]*gate_l + gate[1]*emb_r

Sharding: data-parallel over B across 8 cores; ce/wvec tables replicated.

The binding constraint is the Pool engine: every indirect-DMA gather costs
~1us of SWDGE descriptor-generation time regardless of payload, and
multi-index offset APs corrupt/crash on HW through this runtime, so one
gather instruction moves at most 128 rows (one per partition).  The win
here over the naive per-slot loop: queries are SORTED by their valid
context counts (llen, rlen) on the host and dealt to (core, tile) slots so
that each tile only issues gather instructions for slots that are valid
for at least one of its 128 queries.  Sorted-contiguous dealing gives each
program tile j a shared slot budget (bl[j], br[j]) across all 8 cores;
slots beyond the budget are pad (-1) for every query in the tile, carry an
additive -1e30 mask, and are never gathered (instruction skipped).  This
cuts gather instructions from 35/tile to 1+2*(bl+br) with bl+br ~ 11.5.

Device algorithm per core (2048 queries, 16 tiles of 128, chunks of 4):
  - normalization folded into scores: score = (ctx_raw.u_raw) * rctx * rq.
  - scores bounded so softmax needs no max-shift; exp(-1e30) == 0 exactly.
  - phase1/tile: gather 1+bl+br ce rows/query; PE transposes q, computes
    u = [qT]^T @ [lW^T | rW^T]; DVE dot products; ACT squares; DVE reduces
    row sum-squares.
  - phase2/chunk: softmax + gate on [128, 4*17] staging; l/r slot ranges
    vary per tile (l at 0..bl, r at bl..bl+br) so the per-side reductions
    are per-tile; the gate logit difference uses a host-packed per-tile
    selection of gL_w columns (device computes row1-row0 and replicates).
  - phase3/tile: gather bl+br wvec rows/query (pad slots clamped to row 0,
    weight exactly 0), then scalar*tensor+tensor accumulation on DVE.
"""

from contextlib import ExitStack

import numpy as np

import concourse.bacc as bacc
import concourse.bass as bass
import concourse.mybir as mybir
import concourse.tile as tile
from concourse.bass_utils import run_bass_kernel_spmd
from concourse.masks import make_identity

# Problem shapes (hardcoded per contest contract).
P = 128
CD = 100          # char-embedding dim
L, R = 7, 10
K = L + R         # 17 context slots per query
KQ = K + 1        # + the query row itself
NCE = 200000      # ce table rows
V = 200000        # wvec table rows
WD = 300          # word-vector dim
B = 16384
N_CORES = 8
BC = B // N_CORES     # 2048 queries per core
NT = BC // P          # 16 tiles of 128 queries
CHUNK = 4             # tiles per phase chunk

F32 = mybir.dt.float32
I32 = mybir.dt.int32
Alu = mybir.AluOpType
Act = mybir.ActivationFunctionType
Ax = mybir.AxisListType


def _build_nc(bl, br):
    """bl/br: per-tile l/r slot budgets (length NT tuples)."""
    nc = bacc.Bacc("TRN2", target_bir_lowering=False, debug=False,
                   num_devices=N_CORES)

    ce = nc.dram_tensor("ce_raw", [NCE, CD], F32, kind="ExternalInput")
    wv = nc.dram_tensor("wvec", [V, WD], F32, kind="ExternalInput")
    lW = nc.dram_tensor("lW", [CD, CD], F32, kind="ExternalInput")
    rW = nc.dram_tensor("rW", [CD, CD], F32, kind="ExternalInput")
    gsel = nc.dram_tensor("gsel", [2, NT * K], F32, kind="ExternalInput")
    gb = nc.dram_tensor("gL_b", [2], F32, kind="ExternalInput")
    # host-packed per-(partition, tile, slot) index tensors
    cidx_d = nc.dram_tensor("cidx", [P, NT, KQ], I32, kind="ExternalInput")
    widx_d = nc.dram_tensor("widx", [P, NT, K], I32, kind="ExternalInput")
    out = nc.dram_tensor("out", [BC, WD], F32, kind="ExternalOutput")

    with tile.TileContext(nc) as tc, ExitStack() as ctx:
        consts = ctx.enter_context(tc.tile_pool(name="consts", bufs=1))
        stage = ctx.enter_context(tc.tile_pool(name="stage", bufs=1))
        cep = ctx.enter_context(tc.tile_pool(name="cep", bufs=4))
        wvp = ctx.enter_context(tc.tile_pool(name="wvp", bufs=3))
        work = ctx.enter_context(tc.tile_pool(name="work", bufs=2))
        psum = ctx.enter_context(tc.tile_pool(name="psum", bufs=2, space="PSUM"))

        # ---------------- constants ----------------
        identity = consts.tile([P, P], F32)
        make_identity(nc, identity[:, :])

        # W^T for both sides packed as [100, 0:100]=lW^T, [100, 100:200]=rW^T
        wt_both = consts.tile([P, 2 * CD], F32)
        for side, wdram in enumerate((lW, rW)):
            wl = consts.tile([P, P], F32, name=f"wload{side}")
            nc.sync.dma_start(out=wl[0:CD, 0:CD], in_=wdram[:, :])
            wt_ps = psum.tile([P, P], F32, name=f"wt_ps{side}", tag="wt_ps")
            nc.tensor.transpose(
                out=wt_ps[0:CD, 0:CD], in_=wl[0:CD, 0:CD],
                identity=identity[0:CD, 0:CD])
            nc.vector.tensor_copy(
                out=wt_both[0:CD, side * CD:(side + 1) * CD],
                in_=wt_ps[0:CD, 0:CD])

        # per-tile gate-weight difference, replicated across partitions:
        # gwd[p, t*K+s] = gsel[1, t*K+s] - gsel[0, t*K+s]
        gselrow = consts.tile([1, 2 * NT * K + 2], F32)
        nc.sync.dma_start(out=gselrow[0:1, 0:2 * NT * K], in_=gsel[:, :])
        nc.sync.dma_start(
            out=gselrow[0:1, 2 * NT * K:2 * NT * K + 2], in_=gb[:])
        gdrow = consts.tile([1, NT * K + 1], F32)
        nc.vector.tensor_tensor(
            out=gdrow[0:1, 0:NT * K], in0=gselrow[0:1, NT * K:2 * NT * K],
            in1=gselrow[0:1, 0:NT * K], op=Alu.subtract)
        nc.vector.tensor_tensor(
            out=gdrow[0:1, NT * K:NT * K + 1],
            in0=gselrow[0:1, 2 * NT * K + 1:2 * NT * K + 2],
            in1=gselrow[0:1, 2 * NT * K:2 * NT * K + 1], op=Alu.subtract)
        ones1 = consts.tile([1, P], F32)
        nc.vector.memset(ones1[:, :], 1.0)
        gwdrep = consts.tile([P, NT, K], F32)
        gbd = consts.tile([P, 1], F32)
        # replicate in two matmuls (PSUM bank is 512 f32 wide)
        half = (NT // 2) * K
        for h in range(2):
            cols = slice(h * half, (h + 1) * half)
            rep_ps = psum.tile([P, half], F32, name=f"rep{h}", tag="rep_ps")
            nc.tensor.matmul(out=rep_ps[:, :], lhsT=ones1[0:1, :],
                             rhs=gdrow[0:1, cols], start=True, stop=True)
            nc.vector.tensor_copy(
                out=gwdrep[:, h * (NT // 2):(h + 1) * (NT // 2), :]
                    .rearrange("p a b -> p (a b)"),
                in_=rep_ps[:, :])
        rep_ps = psum.tile([P, 1], F32, name="repb", tag="rep_ps")
        nc.tensor.matmul(out=rep_ps[:, :], lhsT=ones1[0:1, :],
                         rhs=gdrow[0:1, NT * K:NT * K + 1],
                         start=True, stop=True)
        nc.vector.tensor_copy(out=gbd[:, :], in_=rep_ps[:, :])

        # ---------------- index staging ----------------
        cidx = stage.tile([P, NT, KQ], I32)
        nc.sync.dma_start(out=cidx[:, :, :], in_=cidx_d[:, :, :])
        widx = stage.tile([P, NT, K], I32)
        nc.sync.dma_start(out=widx[:, :, :], in_=widx_d[:, :, :])

        # clamped indices (pad -> row 0; weight is exactly 0 downstream)
        ccl = stage.tile([P, NT, KQ], I32)
        nc.vector.tensor_scalar(out=ccl[:, :, :], in0=cidx[:, :, :],
                                scalar1=0, scalar2=None, op0=Alu.max)
        wcl = stage.tile([P, NT, K], I32)
        nc.vector.tensor_scalar(out=wcl[:, :, :], in0=widx[:, :, :],
                                scalar1=0, scalar2=None, op0=Alu.max)

        # additive score mask: 0 for valid slots, -1e30 where index == -1
        maskf = stage.tile([P, NT, KQ], F32)
        nc.vector.tensor_copy(out=maskf[:, :, :], in_=cidx[:, :, :])
        maskt = stage.tile([P, NT, KQ], F32)
        nc.vector.tensor_scalar(out=maskt[:, :, :], in0=maskf[:, :, :],
                                scalar1=0.0, scalar2=1e30,
                                op0=Alu.min, op1=Alu.mult)

        # ---------------- staging buffers ----------------
        # dotraw zero / cssq one so never-gathered slots stay finite
        dotraw = stage.tile([P, NT, K], F32)
        nc.vector.memset(dotraw[:, :, :], 0.0)
        cssq = stage.tile([P, NT, K], F32)
        nc.vector.memset(cssq[:, :, :], 1.0)
        qssq = stage.tile([P, NT], F32)
        expv = stage.tile([P, NT, K], F32)
        wall = stage.tile([P, NT, K], F32)

        def phase1_tile(t):
            nk = 1 + bl[t] + br[t]     # gathered ce rows per query
            ceg = cep.tile([P, KQ, CD], F32, name="ceg", tag="ceg")
            for s in range(nk):
                nc.gpsimd.indirect_dma_start(
                    out=ceg[:, s, :], out_offset=None,
                    in_=ce[:, :],
                    in_offset=bass.IndirectOffsetOnAxis(
                        ap=ccl[:, t, s:s + 1], axis=0),
                )
            # transpose q rows: qt = ceg[:, 0, :]^T  -> [CD, P]
            qt_ps = psum.tile([P, P], F32, name="qt_ps", tag="qt_ps")
            nc.tensor.transpose(out=qt_ps[0:CD, :], in_=ceg[:, 0, :],
                                identity=identity[:, :])
            qt_sb = work.tile([P, P], F32, name="qt_sb", tag="qt_sb")
            nc.scalar.copy(out=qt_sb[0:CD, :], in_=qt_ps[0:CD, :])

            # u[q, 0:100] = lW @ q ; u[q, 100:200] = rW @ q
            u_ps = psum.tile([P, 2 * CD], F32, name="u_ps", tag="u_ps")
            nc.tensor.matmul(out=u_ps[:, :], lhsT=qt_sb[0:CD, :],
                             rhs=wt_both[0:CD, :], start=True, stop=True)

            # dot products: prod[p,k,c] = ctx[p,k,c] * u_side(k)[p,c]
            # l slots at 1..1+bl, r slots at 1+bl..nk
            prod = work.tile([P, K, CD], F32, name="prod", tag="prod")
            nc.vector.tensor_tensor(
                out=prod[:, 0:bl[t], :], in0=ceg[:, 1:1 + bl[t], :],
                in1=u_ps[:, 0:CD].unsqueeze(1).broadcast_to((P, bl[t], CD)),
                op=Alu.mult)
            nc.vector.tensor_tensor(
                out=prod[:, bl[t]:nk - 1, :], in0=ceg[:, 1 + bl[t]:nk, :],
                in1=u_ps[:, CD:2 * CD].unsqueeze(1)
                    .broadcast_to((P, br[t], CD)),
                op=Alu.mult)
            nc.vector.tensor_reduce(out=dotraw[:, t, 0:nk - 1],
                                    in_=prod[:, 0:nk - 1, :],
                                    axis=Ax.X, op=Alu.add)

            # row sum-squares: ACT squares the gathered block, DVE reduces
            sq = work.tile([P, KQ, CD], F32, name="sq", tag="sq")
            nc.scalar.activation(out=sq[:, 0:nk, :], in_=ceg[:, 0:nk, :],
                                 func=Act.Square)
            nc.vector.tensor_reduce(out=qssq[:, t:t + 1], in_=sq[:, 0, :],
                                    axis=Ax.X, op=Alu.add)
            nc.vector.tensor_reduce(out=cssq[:, t, 0:nk - 1],
                                    in_=sq[:, 1:nk, :],
                                    axis=Ax.X, op=Alu.add)

        def phase2_chunk(t0, t1):
            n = t1 - t0
            ts = slice(t0, t1)
            # 1/max(||row||, 1e-12) for q and ctx rows
            sq_t = stage.tile([P, n], F32, name=f"sqt{t0}")
            nc.scalar.activation(out=sq_t[:, :], in_=qssq[:, ts],
                                 func=Act.Sqrt)
            nc.vector.tensor_scalar(out=sq_t[:, :], in0=sq_t[:, :],
                                    scalar1=1e-12, scalar2=None, op0=Alu.max)
            rq = stage.tile([P, n], F32, name=f"rq{t0}")
            nc.vector.reciprocal(out=rq[:, :], in_=sq_t[:, :])

            csq_t = stage.tile([P, n, K], F32, name=f"csqt{t0}")
            nc.scalar.activation(out=csq_t[:, :, :], in_=cssq[:, ts, :],
                                 func=Act.Sqrt)
            nc.vector.tensor_scalar(out=csq_t[:, :, :], in0=csq_t[:, :, :],
                                    scalar1=1e-12, scalar2=None, op0=Alu.max)
            rctx = stage.tile([P, n, K], F32, name=f"rctx{t0}")
            nc.vector.reciprocal(out=rctx[:, :, :], in_=csq_t[:, :, :])

            scr = stage.tile([P, n, K], F32, name=f"scr{t0}")
            nc.vector.tensor_tensor(out=scr[:, :, :], in0=dotraw[:, ts, :],
                                    in1=rctx[:, :, :], op=Alu.mult)
            nc.vector.tensor_tensor(
                out=scr[:, :, :], in0=scr[:, :, :],
                in1=rq[:, :].unsqueeze(2).broadcast_to((P, n, K)),
                op=Alu.mult)
            nc.vector.tensor_tensor(out=scr[:, :, :], in0=scr[:, :, :],
                                    in1=maskt[:, ts, 1:KQ], op=Alu.add)

            nc.scalar.activation(out=expv[:, ts, :], in_=scr[:, :, :],
                                 func=Act.Exp)

            # per-tile l/r reductions (slot ranges vary with budgets)
            sum_l = stage.tile([P, n], F32, name=f"suml{t0}")
            sum_r = stage.tile([P, n], F32, name=f"sumr{t0}")
            gmul = stage.tile([P, n, K], F32, name=f"gmul{t0}")
            nc.vector.tensor_tensor(out=gmul[:, :, :], in0=expv[:, ts, :],
                                    in1=gwdrep[:, ts, :], op=Alu.mult)
            ddl = stage.tile([P, n], F32, name=f"ddl{t0}")
            ddr = stage.tile([P, n], F32, name=f"ddr{t0}")
            for t in range(t0, t1):
                i = t - t0
                nc.vector.tensor_reduce(
                    out=sum_l[:, i:i + 1], in_=expv[:, t, 0:bl[t]],
                    axis=Ax.X, op=Alu.add)
                nc.vector.tensor_reduce(
                    out=sum_r[:, i:i + 1],
                    in_=expv[:, t, bl[t]:bl[t] + br[t]],
                    axis=Ax.X, op=Alu.add)
                nc.vector.tensor_reduce(
                    out=ddl[:, i:i + 1], in_=gmul[:, i, 0:bl[t]],
                    axis=Ax.X, op=Alu.add)
                nc.vector.tensor_reduce(
                    out=ddr[:, i:i + 1], in_=gmul[:, i, bl[t]:bl[t] + br[t]],
                    axis=Ax.X, op=Alu.add)
            rs_l = stage.tile([P, n], F32, name=f"rsl{t0}")
            nc.vector.reciprocal(out=rs_l[:, :], in_=sum_l[:, :])
            rs_r = stage.tile([P, n], F32, name=f"rsr{t0}")
            nc.vector.reciprocal(out=rs_r[:, :], in_=sum_r[:, :])

            # gate logit difference dz = ddl*rs_l + ddr*rs_r + (gb1-gb0)
            m1 = stage.tile([P, n], F32, name=f"m1{t0}")
            nc.vector.tensor_tensor(out=m1[:, :], in0=ddl[:, :],
                                    in1=rs_l[:, :], op=Alu.mult)
            m2 = stage.tile([P, n], F32, name=f"m2{t0}")
            nc.vector.tensor_tensor(out=m2[:, :], in0=ddr[:, :],
                                    in1=rs_r[:, :], op=Alu.mult)
            dz = stage.tile([P, n], F32, name=f"dz{t0}")
            nc.vector.tensor_tensor(out=dz[:, :], in0=m1[:, :], in1=m2[:, :],
                                    op=Alu.add)
            nc.vector.tensor_scalar(out=dz[:, :], in0=dz[:, :],
                                    scalar1=gbd[:, 0:1], scalar2=None,
                                    op0=Alu.add)

            e1 = stage.tile([P, n], F32, name=f"e1{t0}")
            nc.scalar.activation(out=e1[:, :], in_=dz[:, :], func=Act.Exp)
            den = stage.tile([P, n], F32, name=f"den{t0}")
            nc.vector.tensor_scalar(out=den[:, :], in0=e1[:, :], scalar1=1.0,
                                    scalar2=None, op0=Alu.add)
            rden = stage.tile([P, n], F32, name=f"rden{t0}")
            nc.vector.reciprocal(out=rden[:, :], in_=den[:, :])

            # c_l = g0*rs_l = rs_l/(1+e1); c_r = g1*rs_r = rs_r*e1/(1+e1)
            c_l = stage.tile([P, n], F32, name=f"cl{t0}")
            nc.vector.tensor_tensor(out=c_l[:, :], in0=rs_l[:, :],
                                    in1=rden[:, :], op=Alu.mult)
            c_r = stage.tile([P, n], F32, name=f"cr{t0}")
            nc.vector.tensor_tensor(out=c_r[:, :], in0=rs_r[:, :],
                                    in1=rden[:, :], op=Alu.mult)
            nc.vector.tensor_tensor(out=c_r[:, :], in0=c_r[:, :],
                                    in1=e1[:, :], op=Alu.mult)

            # final per-slot weights (per-tile ranges)
            for t in range(t0, t1):
                i = t - t0
                nc.vector.tensor_tensor(
                    out=wall[:, t, 0:bl[t]], in0=expv[:, t, 0:bl[t]],
                    in1=c_l[:, i:i + 1].broadcast_to((P, bl[t])),
                    op=Alu.mult)
                nc.vector.tensor_tensor(
                    out=wall[:, t, bl[t]:bl[t] + br[t]],
                    in0=expv[:, t, bl[t]:bl[t] + br[t]],
                    in1=c_r[:, i:i + 1].broadcast_to((P, br[t])),
                    op=Alu.mult)

        def phase3_tile(t):
            nw = bl[t] + br[t]
            wvg = wvp.tile([P, K, WD], F32, name="wvg", tag="wvg")
            for s in range(nw):
                nc.gpsimd.indirect_dma_start(
                    out=wvg[:, s, :], out_offset=None,
                    in_=wv[:, :],
                    in_offset=bass.IndirectOffsetOnAxis(
                        ap=wcl[:, t, s:s + 1], axis=0),
                )
            acc_a = work.tile([P, WD], F32, name="acc_a", tag="acc_a")
            acc_b = work.tile([P, WD], F32, name="acc_b", tag="acc_b")
            nc.vector.tensor_scalar(out=acc_a[:, :], in0=wvg[:, 0, :],
                                    scalar1=wall[:, t, 0:1], scalar2=None,
                                    op0=Alu.mult)
            for s in range(1, nw):
                src, dst = (acc_a, acc_b) if s % 2 == 1 else (acc_b, acc_a)
                nc.vector.scalar_tensor_tensor(
                    out=dst[:, :], in0=wvg[:, s, :],
                    scalar=wall[:, t, s:s + 1], in1=src[:, :],
                    op0=Alu.mult, op1=Alu.add)
            res = acc_a if (nw - 1) % 2 == 0 else acc_b
            nc.sync.dma_start(out=out[t * P:(t + 1) * P, :], in_=res[:, :])

        for c0 in range(0, NT, CHUNK):
            for t in range(c0, c0 + CHUNK):
                phase1_tile(t)
            phase2_chunk(c0, c0 + CHUNK)
            for t in range(c0, c0 + CHUNK):
                phase3_tile(t)

    nc.compile()
    return nc


_NC_CACHE = {}


def _get_nc(bl, br):
    key = (tuple(bl), tuple(br))
    if key not in _NC_CACHE:
        _NC_CACHE[key] = _build_nc(tuple(bl), tuple(br))
    return _NC_CACHE[key]


def _maxflow(combos, buds, cap):
    """Max-flow combos -> budget groups (scipy). Returns (flow_value,
    assign) where assign[(i, g)] = queries of combo i routed to group g."""
    from scipy.sparse import csr_matrix
    from scipy.sparse.csgraph import maximum_flow
    C, G = len(combos), len(buds)
    n = C + G + 2
    src, snk = 0, n - 1
    rows, cols, data = [], [], []
    for i, (l, r, cnt) in enumerate(combos):
        rows.append(src)
        cols.append(1 + i)
        data.append(cnt)
        for g, (bl, br) in enumerate(buds):
            if l <= bl and r <= br:
                rows.append(1 + i)
                cols.append(1 + C + g)
                data.append(cnt)
    for g in range(G):
        rows.append(1 + C + g)
        cols.append(snk)
        data.append(cap)
    graph = csr_matrix((data, (rows, cols)), shape=(n, n), dtype=np.int32)
    res = maximum_flow(graph, src, snk)
    assign = {}
    fl = res.flow.tocoo()
    for u, v, f in zip(fl.row, fl.col, fl.data):
        if f > 0 and 1 <= u <= C and C < v < n - 1:
            assign[(u - 1, v - 1 - C)] = int(f)
    return int(res.flow_value), assign


_FEAS_CACHE = {}


def _feasible(combos, buds, cap, total):
    key = tuple(sorted(buds))
    hit = _FEAS_CACHE.get(key)
    if hit is None:
        hit = _maxflow(combos, buds, cap)[0] == total
        _FEAS_CACHE[key] = hit
    return hit


def _plan(lixs_c, rixs_c):
    """Assign queries to (core, tile, partition) slots so that each program
    tile j has minimal shared slot budgets (bl[j], br[j]).  All 8 cores run
    one program, so budgets are shared: group j = 8 cores x 128 partitions
    = 1024 queries.  The budget design is optimized by local search with a
    max-flow feasibility check over the (llen, rlen) combo counts."""
    llen = (lixs_c != -1).sum(1)                      # [B]
    rlen = (rixs_c != -1).sum(1)
    cap = N_CORES * P                                 # queries per group
    cnt = {}
    for l, r in zip(llen.tolist(), rlen.tolist()):
        cnt[(l, r)] = cnt.get((l, r), 0) + 1
    combos = [(l, r, c) for (l, r), c in sorted(cnt.items())]

    # start from snake-sorted contiguous groups (always feasible)
    key = (llen - 1) * 1000 + np.where(llen % 2 == 1, rlen, R + 1 - rlen)
    order = np.argsort(key, kind="stable")
    grp = order.reshape(NT, cap)
    buds = list(zip(llen[grp].max(1).tolist(), rlen[grp].max(1).tolist()))

    def cost(b):
        return sum(x + y for x, y in b)

    improved = True
    while improved:
        improved = False
        for g in range(NT):
            for di in (0, 1):
                bl, br = buds[g]
                nb = (bl - 1, br) if di == 0 else (bl, br - 1)
                if nb[0] < 1 or nb[1] < 1:
                    continue
                trial = list(buds)
                trial[g] = nb
                if _feasible(combos, trial, cap, len(llen)):
                    buds = trial
                    improved = True
        if not improved:
            done = False
            for g1 in range(NT):
                for di1 in (0, 1):
                    bl, br = buds[g1]
                    up = (bl + (di1 == 0), br + (di1 == 1))
                    if up[0] > L or up[1] > R:
                        continue
                    base = list(buds)
                    base[g1] = up
                    for g2 in range(NT):
                        for di2 in (0, 1):
                            for g3 in range(NT):
                                for di3 in (0, 1):
                                    trial = [list(x) for x in base]
                                    trial[g2][di2] -= 1
                                    trial[g3][di3] -= 1
                                    if min(trial[g2]) < 1 or min(trial[g3]) < 1:
                                        continue
                                    t2 = [tuple(x) for x in trial]
                                    if cost(t2) >= cost(buds):
                                        continue
                                    if _feasible(combos, t2, cap, len(llen)):
                                        buds = t2
                                        done = improved = True
                                        break
                                if done:
                                    break
                            if done:
                                break
                        if done:
                            break
                    if done:
                        break
                if done:
                    break

    # materialize the assignment from the final flow
    flow, assign = _maxflow(combos, buds, cap)
    assert flow == len(llen)
    by_combo = {}
    for i, (l, r, c) in enumerate(combos):
        ids = np.where((llen == l) & (rlen == r))[0]
        by_combo[i] = list(ids)
    groups = [[] for _ in range(NT)]
    for (i, g), f in sorted(assign.items()):
        take, by_combo[i] = by_combo[i][:f], by_combo[i][f:]
        groups[g].extend(take)
    # order groups by ascending cost (cosmetic/stable)
    gorder = sorted(range(NT), key=lambda g: (buds[g][0] + buds[g][1],
                                              buds[g]))
    bl = [buds[g][0] for g in gorder]
    br = [buds[g][1] for g in gorder]
    bsel = np.empty((N_CORES, NT, P), dtype=np.int64)
    for j, g in enumerate(gorder):
        arr = np.asarray(groups[g])
        assert arr.size == cap
        bsel[:, j, :] = arr.reshape(N_CORES, P)
    return bsel, bl, br


def prepare(inputs):
    """Plan, build the module and the per-core input maps (no run)."""
    inp = {k: np.asarray(v) for k, v in inputs.items()}
    bsel, bl, br = _plan(inp["lixs_c"], inp["rixs_c"])
    nc = _get_nc(bl, br)

    # per-tile selection of gL_w columns: l slots 0..bl-1 -> cols 0..bl-1,
    # r slots bl..bl+br-1 -> cols L..L+br-1, rest zero.
    gw = inp["gL_w"]
    gsel = np.zeros((2, NT, K), dtype=np.float32)
    for j in range(NT):
        gsel[:, j, 0:bl[j]] = gw[:, 0:bl[j]]
        gsel[:, j, bl[j]:bl[j] + br[j]] = gw[:, L:L + br[j]]
    gsel = np.ascontiguousarray(gsel.reshape(2, NT * K))

    shared = {k: inp[k] for k in ("ce_raw", "wvec", "lW", "rW", "gL_b")}
    shared["gsel"] = gsel
    in_maps = []
    for c in range(N_CORES):
        sel = bsel[c]                                  # [NT, P] query ids
        cidx = np.full((NT, P, KQ), -1, dtype=np.int32)
        cidx[:, :, 0] = inp["qidx"][sel]
        widx = np.full((NT, P, K), -1, dtype=np.int32)
        lc, rc = inp["lixs_c"][sel], inp["rixs_c"][sel]   # [NT, P, L/R]
        lw, rw = inp["lixs_w"][sel], inp["rixs_w"][sel]
        for j in range(NT):
            cidx[j, :, 1:1 + bl[j]] = lc[j, :, 0:bl[j]]
            cidx[j, :, 1 + bl[j]:1 + bl[j] + br[j]] = rc[j, :, 0:br[j]]
            widx[j, :, 0:bl[j]] = lw[j, :, 0:bl[j]]
            widx[j, :, bl[j]:bl[j] + br[j]] = rw[j, :, 0:br[j]]
        m = dict(shared)
        m["cidx"] = np.ascontiguousarray(cidx.transpose(1, 0, 2))
        m["widx"] = np.ascontiguousarray(widx.transpose(1, 0, 2))
        in_maps.append(m)
    return in_maps, nc, bsel


def kernel(**inputs):
    in_maps, nc, bsel = prepare(inputs)
    res = run_bass_kernel_spmd(nc, in_maps, list(range(N_CORES)))
    out = np.empty((B, WD), dtype=np.float32)
    for c in range(N_CORES):
        # device row j*P+p holds query bsel[c, j, p]
        out[bsel[c].reshape(-1)] = res.results[c]["out"]
    return out


# for test.py's cost-model timing fallback
def _last_nc():
    return next(iter(_NC_CACHE.values())) if _NC_CACHE else None
